# revision 34
# baseline (speedup 1.0000x reference)
"""BiMamba block kernel for 8 TRN2 NeuronCores.

Sharding: cores 0-3 run the fwd Mamba pass, cores 4-7 the bwd pass (on
time-reversed x). Within each 4-core group, d_inner (2048) is sharded
into 4 slices of 512 channels. out_proj and the fusion matmul are folded
into one [512, 1024] weight per core; each core returns its d_inner-shard
partial of [B*L, DM] and the 4-way group sum happens on host during
gather/unshard (with the fwd+bwd+bias combine).

Layout on device is feature-major [d, t]: channels in partitions, time in
the free dimension, so the selective scan maps onto tensor_tensor_scan
(one recurrence per partition lane, scanned along free/time).

Precision: bf16 throughout (data, delta, scan-block tensors, and the y
accumulator); PSUM accumulation is fp32 by HW. bf16 delta/acc were
validated against the fp32 variants (rel err 6.25e-3 either way): delta
errors are independent per step so the accumulated decay-exponent error
stays ~0.4% of an O(1) sum.

Scheduling notes (measured on HW):
- DVE tensor_tensor bf16 all-SBUF runs in 2x mode (~0.61 ns/free-elem);
  tensor_tensor_scan runs ~2.1 ns/free-elem and has no fast mode.
- Concurrent GpSimd compute halves DVE scan throughput via SBUF port
  contention (8.08us vs 4.33us per [128,2048] scan), so GpSimd issues
  only collectives. DMA traffic does not contend.
- Packing b1's prep (conv/x_proj/AllReduce/softplus) inside b0's scan
  window also slows every op ~15% (SBUF bandwidth saturation); the
  pipeline instead overlaps only matmul-side work (phase1_mm, zrecomp)
  with the scans and runs b1 prep between the two scan blocks.
- Per-channel work is pushed onto TensorE as diagonal-weight matmuls:
  the depthwise conv is 4 accumulating diag matmuls per tile (silu then
  reads PSUM directly), and channel-block 0's y accumulation runs as
  identity matmuls into 4 dedicated PSUM banks seeded with diag(D) @ u.
- A burst of dummy matmuls at t=0 spins up the PE p-state during the
  initial weight DMAs so the first in-proj matmuls run at full clock.
"""

import os
import sys

import numpy as np

sys.path.insert(0, "/opt/trn_rl_repo")

B = 2
L = 2048
DM = 1024
DI = 2048
DS = 512          # d_inner shard per core
N = 16            # d_state
R = 64            # dt_rank
NB = DS // 128    # 4 channel blocks of 128 per core
K_CONV = 4

_CACHE = {}


def build_program(data_dtype="bfloat16", scan_dtype="bfloat16", acc_dtype="float32"):
    from concourse import bacc, mybir, tile

    F32 = mybir.dt.float32
    DDT = getattr(mybir.dt, data_dtype)   # matmul inputs / data tensors
    SDT = getattr(mybir.dt, scan_dtype)   # scan-block tensors (dA, dBu, h, p)
    ADT = getattr(mybir.dt, acc_dtype)    # y accumulator
    ALU = mybir.AluOpType
    ACT = mybir.ActivationFunctionType

    nc = bacc.Bacc(
        "TRN2", target_bir_lowering=False, debug=False, num_devices=8
    )

    # ---- external inputs (per-core, host-prepped) ----
    xT_d = nc.dram_tensor("xT", [B, DM, L], DDT, kind="ExternalInput")
    w_xs_d = nc.dram_tensor("w_xs", [DM, DS], DDT, kind="ExternalInput")
    w_z_d = nc.dram_tensor("w_z", [DM, DS], DDT, kind="ExternalInput")
    w_xp_d = nc.dram_tensor("w_xp", [DS, 96], DDT, kind="ExternalInput")
    w_dt_d = nc.dram_tensor("w_dt", [R, DS], DDT, kind="ExternalInput")
    w_out_d = nc.dram_tensor("w_out", [DS, DM], DDT, kind="ExternalInput")
    conv_wd_d = nc.dram_tensor(
        "conv_wd", [NB * K_CONV * 128, 128], DDT, kind="ExternalInput"
    )
    conv_b_d = nc.dram_tensor("conv_b", [128, NB], F32, kind="ExternalInput")
    dtb_d = nc.dram_tensor("dtb", [128, NB], F32, kind="ExternalInput")
    dskip_d = nc.dram_tensor("dskip", [128, NB], F32, kind="ExternalInput")
    a_pack_d = nc.dram_tensor("a_pack", [128, NB * N], F32, kind="ExternalInput")
    idsk_d = nc.dram_tensor("idsk", [256, 128], DDT, kind="ExternalInput")

    out_d = nc.dram_tensor("out", [B * L, DM], DDT, kind="ExternalOutput")

    # ---- internal dram ----
    xdbl_loc = nc.dram_tensor("xdbl_loc", [B, 2, 96, L // 2], DDT)
    xdbl_red = nc.dram_tensor("xdbl_red", [B, 2, 96, L // 2], DDT)
    bc_bf = nc.dram_tensor("bc_bf", [B, 2 * N, L], DDT)

    GROUPS = [[0, 1, 2, 3], [4, 5, 6, 7]]

    with tile.TileContext(nc) as tc:
        with (
            tc.tile_pool(name="const", bufs=1) as cpool,
            tc.tile_pool(name="resid", bufs=1) as rpool,
            tc.tile_pool(name="work", bufs=2) as wpool,
            tc.tile_pool(name="scan", bufs=2) as spool,
            tc.tile_pool(name="psum", bufs=2, space="PSUM") as ppool,
            tc.tile_pool(name="psum_o", bufs=2, space="PSUM") as opool,
            tc.tile_pool(name="psum_a", bufs=1, space="PSUM") as apool,
        ):
            # ---- load weights/constants once ----
            w_xs_sb = cpool.tile([128, 8 * DS], DDT, tag="wxs")
            w_z_sb = cpool.tile([128, 8 * DS], DDT, tag="wz")
            for mt in range(8):
                nc.sync.dma_start(
                    out=w_xs_sb[:, mt * DS:(mt + 1) * DS],
                    in_=w_xs_d.ap()[mt * 128:(mt + 1) * 128, :],
                )
                nc.sync.dma_start(
                    out=w_z_sb[:, mt * DS:(mt + 1) * DS],
                    in_=w_z_d.ap()[mt * 128:(mt + 1) * 128, :],
                )
            w_xp_sb = cpool.tile([128, NB * 96], DDT, tag="wxp")
            for j in range(NB):
                nc.sync.dma_start(
                    out=w_xp_sb[:, j * 96:(j + 1) * 96],
                    in_=w_xp_d.ap()[j * 128:(j + 1) * 128, :],
                )
            w_dt_sb = cpool.tile([R, DS], DDT, tag="wdt")
            nc.sync.dma_start(out=w_dt_sb[:, :], in_=w_dt_d.ap()[:, :])
            w_out_sb = cpool.tile([128, NB * DM], DDT, tag="wout")
            for j in range(NB):
                nc.sync.dma_start(
                    out=w_out_sb[:, j * DM:(j + 1) * DM],
                    in_=w_out_d.ap()[j * 128:(j + 1) * 128, :],
                )
            # depthwise-conv taps as diagonal [128,128] matrices so the
            # conv runs as 4 accumulating matmuls on TensorE (idle capacity)
            # instead of 4 DVE ops; block order is (j, k).
            conv_wd_sb = cpool.tile([128, NB * K_CONV * 128], DDT, tag="convwd")
            for blk in range(NB * K_CONV):
                nc.sync.dma_start(
                    out=conv_wd_sb[:, blk * 128:(blk + 1) * 128],
                    in_=conv_wd_d.ap()[blk * 128:(blk + 1) * 128, :],
                )
            conv_b_sb = cpool.tile([128, NB], F32, tag="convb")
            nc.sync.dma_start(out=conv_b_sb[:, :], in_=conv_b_d.ap()[:, :])
            dtb_sb = cpool.tile([128, NB], F32, tag="dtb")
            nc.sync.dma_start(out=dtb_sb[:, :], in_=dtb_d.ap()[:, :])
            dskip_sb = cpool.tile([128, NB], F32, tag="dskip")
            nc.sync.dma_start(out=dskip_sb[:, :], in_=dskip_d.ap()[:, :])
            a_sb = cpool.tile([128, NB * N], F32, tag="apack")
            nc.sync.dma_start(out=a_sb[:, :], in_=a_pack_d.ap()[:, :])
            # [identity | diag(D[j=0])] for the j=0 PSUM accumulation path
            idsk_sb = cpool.tile([128, 256], DDT, tag="idsk")
            nc.sync.dma_start(out=idsk_sb[:, 0:128], in_=idsk_d.ap()[0:128, :])
            nc.sync.dma_start(out=idsk_sb[:, 128:256],
                              in_=idsk_d.ap()[128:256, :])

            # spin up the PE clock (p-state ramp) with dummy matmuls
            # during the initial weight/xT DMA window so the first real
            # in-proj matmuls run at full frequency.
            warm_sb = cpool.tile([128, 128], DDT, tag="warm")
            nc.vector.memset(warm_sb[:, :], 0.0)
            for _ in range(12):
                warm_ps = ppool.tile([128, 512], F32, tag="mm")
                for r in range(2):
                    nc.tensor.matmul(
                        out=warm_ps[:, 0:128],
                        lhsT=warm_sb[:, :],
                        rhs=warm_sb[:, :],
                        start=(r == 0),
                        stop=(r == 1),
                    )

            st = [{} for _ in range(B)]

            def phase1_mm(b, tchs=(0, 1, 2, 3)):
                """in-proj matmuls only (PE/ScalarE) -> xs."""
                s = st[b]
                if 0 in tchs:
                    xs_sb = rpool.tile([128, NB * (L + 3)], DDT, tag="xs")
                    s["xs"] = xs_sb
                    for j in range(NB):
                        nc.vector.memset(
                            xs_sb[:, j * (L + 3):j * (L + 3) + 3], 0.0
                        )
                xs_sb = s["xs"]
                for tch in tchs:
                    t0 = tch * 512
                    xt_sb = wpool.tile([128, 8 * 512], DDT, tag="xt", bufs=1)
                    for mt in range(8):
                        nc.sync.dma_start(
                            out=xt_sb[:, mt * 512:(mt + 1) * 512],
                            in_=xT_d.ap()[b, mt * 128:(mt + 1) * 128,
                                          t0:t0 + 512],
                        )
                    for j in range(NB):
                        xs_ps = ppool.tile([128, 512], F32, tag="mm")
                        for mt in range(8):
                            nc.tensor.matmul(
                                out=xs_ps[:, :],
                                lhsT=w_xs_sb[:, mt * DS + j * 128:
                                             mt * DS + (j + 1) * 128],
                                rhs=xt_sb[:, mt * 512:(mt + 1) * 512],
                                start=(mt == 0),
                                stop=(mt == 7),
                            )
                        nc.scalar.activation(
                            out=xs_sb[:, j * (L + 3) + 3 + t0:
                                      j * (L + 3) + 3 + t0 + 512],
                            in_=xs_ps[:, :],
                            func=ACT.Copy,
                        )

            HL = L // 2

            def conv_q(b, q):
                """conv+silu -> u for L-quarter q (conv out rows
                [q*512, q*512+512) read xs cols [q*512, q*512+514], which
                the first q+1 in-proj tch chunks plus the 3-pad cover)."""
                s = st[b]
                xs_sb = s["xs"]
                if q == 0:
                    u_sb = rpool.tile([128, NB * L], DDT, tag="u")
                    s["u"] = u_sb
                u_sb = s["u"]
                qb = q * 512
                for j in range(NB):
                    xsj = xs_sb[:, j * (L + 3):(j + 1) * (L + 3)]
                    xc_ps = ppool.tile([128, 512], F32, tag="mm")
                    for k in range(K_CONV):
                        blk = j * K_CONV + k
                        nc.tensor.matmul(
                            out=xc_ps[:, :],
                            lhsT=conv_wd_sb[:, blk * 128:(blk + 1) * 128],
                            rhs=xsj[:, qb + k:qb + k + 512],
                            start=(k == 0),
                            stop=(k == K_CONV - 1),
                        )
                    nc.scalar.activation(
                        out=u_sb[:, j * L + qb:j * L + qb + 512],
                        in_=xc_ps[:, :],
                        func=ACT.Silu,
                        bias=conv_b_sb[:, j:j + 1],
                        scale=1.0,
                    )

            def phase1_rest(b, h):
                """x_proj partial, group AllReduce, B/C row staging for
                L-half h (conv/silu handled by conv_q)."""
                s = st[b]
                u_sb = s["u"]
                hb = h * HL
                xdbl_st = wpool.tile([97, HL], DDT, tag="xc", bufs=1)
                for tch in range(2):
                    t0 = hb + tch * 512
                    xp_ps = ppool.tile([128, 512], F32, tag="mm")
                    for j in range(NB):
                        nc.tensor.matmul(
                            out=xp_ps[0:96, :],
                            lhsT=w_xp_sb[:, j * 96:(j + 1) * 96],
                            rhs=u_sb[:, j * L + t0:j * L + t0 + 512],
                            start=(j == 0),
                            stop=(j == NB - 1),
                        )
                    nc.scalar.activation(
                        out=xdbl_st[0:96, tch * 512:(tch + 1) * 512],
                        in_=xp_ps[0:96, :],
                        func=ACT.Copy,
                    )
                nc.scalar.dma_start(
                    out=xdbl_loc.ap()[b, h, :, :], in_=xdbl_st[0:96, :]
                )
                nc.gpsimd.collective_compute(
                    "AllReduce",
                    mybir.AluOpType.add,
                    replica_groups=GROUPS,
                    ins=[xdbl_loc.ap()[b, h, :, :].opt()],
                    outs=[xdbl_red.ap()[b, h, :, :].opt()],
                )
                bc_b16 = spool.tile([2 * N, HL], DDT, tag="h", bufs=1)
                nc.scalar.dma_start(
                    out=bc_b16[:, :],
                    in_=xdbl_red.ap()[b, h, R:R + 2 * N, :],
                )
                nc.scalar.dma_start(
                    out=bc_bf.ap()[b, :, hb:hb + HL], in_=bc_b16[:, :]
                )

            def prep_delta(b, h):
                """dt_proj + softplus -> delta (exp/ln table block).

                delta is bf16 double-buffered: per-step decay-exponent
                error from bf16 delta is ~0.4% of an O(1) exponent sum
                (errors independent per step), well within the gate; the
                second buffer lets b1's delta build during scan(0)."""
                s = st[b]
                hb = h * HL
                dt_sb = spool.tile([64, HL], DDT, tag="dt", bufs=2)
                nc.scalar.dma_start(
                    out=dt_sb[:, :], in_=xdbl_red.ap()[b, h, 0:64, :]
                )
                if h == 0:
                    delta_sb = rpool.tile(
                        [128, NB * L], DDT, tag="delta", bufs=2
                    )
                    s["delta"] = delta_sb
                delta_sb = s["delta"]
                # softplus(v + b) = ln(1 + exp(v + b)); no softplus table
                # on this compiler build. All Exps first (into the xs tile,
                # dead after conv), then all Lns (emitted via prep_delta_ln):
                # each act-table switch costs ~10us of ACT_TABLE_LOADs.
                xs_sb = s["xs"]
                LP = L + 3
                for j in range(NB):
                    for tch in range(2):
                        t0 = hb + tch * 512
                        dp_ps = ppool.tile([128, 512], F32, tag="mm")
                        nc.tensor.matmul(
                            out=dp_ps[:, :],
                            lhsT=w_dt_sb[:, j * 128:(j + 1) * 128],
                            rhs=dt_sb[:, tch * 512:(tch + 1) * 512],
                            start=True,
                            stop=True,
                        )
                        nc.scalar.activation(
                            out=xs_sb[:, j * LP + 3 + t0:j * LP + 3 + t0 + 512],
                            in_=dp_ps[:, :],
                            func=ACT.Exp,
                            bias=dtb_sb[:, j:j + 1],
                            scale=1.0,
                        )

            def prep_delta_ln_w(b):
                """Ln (softplus finish) and w = delta*u, interleaved per
                channel block so the scan can start as soon as block 0's
                delta/w are ready instead of waiting for all four."""
                s = st[b]
                delta_sb = s["delta"]
                xs_sb = s["xs"]
                u_sb = s["u"]
                w_sb = rpool.tile([128, NB * L], DDT, tag="w")
                s["w"] = w_sb
                LP = L + 3
                for j in range(NB):
                    for t0 in range(0, L, 512):
                        nc.scalar.activation(
                            out=delta_sb[:, j * L + t0:j * L + t0 + 512],
                            in_=xs_sb[:, j * LP + 3 + t0:j * LP + 3 + t0 + 512],
                            func=ACT.Ln,
                            bias=1.0,
                            scale=1.0,
                        )
                    nc.vector.tensor_tensor(
                        out=w_sb[:, j * L:(j + 1) * L],
                        in0=delta_sb[:, j * L:(j + 1) * L],
                        in1=u_sb[:, j * L:(j + 1) * L],
                        op=ALU.mult,
                    )

            def prep_acc(b):
                """acc = D*u (the skip term); must follow prior b's gate."""
                s = st[b]
                u_sb = s["u"]
                acc_sb = rpool.tile([128, NB * L], ADT, tag="acc")
                s["acc"] = acc_sb
                for j in range(1, NB):
                    nc.scalar.activation(
                        out=acc_sb[:, j * L:(j + 1) * L],
                        in_=u_sb[:, j * L:(j + 1) * L],
                        func=ACT.Copy,
                        scale=dskip_sb[:, j:j + 1],
                    )

            def zrecomp(b):
                """z-proj recomputed from re-loaded xT, stored RAW (Copy is
                in every ACT table -> no table thrash inside the scan
                window). DMAs go via ScalarE's queue to bypass the
                scan-paced brep/crep DMAs on sync."""
                s = st[b]
                zsil_sb = rpool.tile([128, NB * L], DDT, tag="zsil")
                s["zsil"] = zsil_sb
                for tch in range(4):
                    t0 = tch * 512
                    xt3_sb = wpool.tile([128, 8 * 512], DDT, tag="xt", bufs=1)
                    for mt in range(8):
                        nc.scalar.dma_start(
                            out=xt3_sb[:, mt * 512:(mt + 1) * 512],
                            in_=xT_d.ap()[b, mt * 128:(mt + 1) * 128,
                                          t0:t0 + 512],
                        )
                    for j in range(NB):
                        z_ps = opool.tile([128, 512], F32, tag="omm")
                        for mt in range(8):
                            nc.tensor.matmul(
                                out=z_ps[:, :],
                                lhsT=w_z_sb[:, mt * DS + j * 128:
                                            mt * DS + (j + 1) * 128],
                                rhs=xt3_sb[:, mt * 512:(mt + 1) * 512],
                                start=(mt == 0),
                                stop=(mt == 7),
                            )
                        nc.scalar.activation(
                            out=zsil_sb[:, j * L + t0:j * L + t0 + 512],
                            in_=z_ps[:, :],
                            func=ACT.Copy,
                        )

            def scan(b, n_range=None):
                s = st[b]
                delta_sb, w_sb, acc_sb = s["delta"], s["w"], s["acc"]
                u_sb = s["u"]
                # j=0's y accumulates in PSUM on TensorE (identity matmuls),
                # seeded with diag(D) @ u; frees ~18us/b of DVE adds.
                acc0 = [None] * 4
                for c in range(4):
                    a0 = apool.tile([128, 512], F32, tag=f"acc0c{c}")
                    acc0[c] = a0
                    nc.tensor.matmul(
                        out=a0[:, :],
                        lhsT=idsk_sb[:, 128:256],
                        rhs=u_sb[:, c * 512:(c + 1) * 512],
                        start=True,
                        stop=False,
                    )
                s["acc0"] = acc0
                for n in (n_range if n_range is not None else range(N)):
                    brep = spool.tile([128, L], DDT, tag="brep")
                    nc.sync.dma_start(
                        out=brep[:, :],
                        in_=bc_bf.ap()[b, n:n + 1, :].partition_broadcast(128),
                    )
                    crep = spool.tile([128, L], DDT, tag="crep", bufs=2)
                    nc.sync.dma_start(
                        out=crep[:, :],
                        in_=bc_bf.ap()[b, N + n:N + n + 1, :]
                        .partition_broadcast(128),
                    )
                    for j in range(NB):
                        dA = spool.tile([128, L], SDT, tag="dA")
                        nc.scalar.activation(
                            out=dA[:, :],
                            in_=delta_sb[:, j * L:(j + 1) * L],
                            func=ACT.Exp,
                            scale=a_sb[:, j * N + n:j * N + n + 1],
                        )
                        dBu = spool.tile([128, L], SDT, tag="dBu", bufs=1)
                        nc.vector.tensor_tensor(
                            out=dBu[:, :],
                            in0=brep[:, :],
                            in1=w_sb[:, j * L:(j + 1) * L],
                            op=ALU.mult,
                        )
                        h = spool.tile([128, L], SDT, tag="h", bufs=1)
                        nc.vector.tensor_tensor_scan(
                            out=h[:, :],
                            data0=dA[:, :],
                            data1=dBu[:, :],
                            initial=0.0,
                            op0=ALU.mult,
                            op1=ALU.add,
                        )
                        p = spool.tile([128, L], SDT, tag="p", bufs=1)
                        nc.vector.tensor_tensor(
                            out=p[:, :],
                            in0=crep[:, :],
                            in1=h[:, :],
                            op=ALU.mult,
                        )
                        if j == 0:
                            for c in range(4):
                                nc.tensor.matmul(
                                    out=s["acc0"][c][:, :],
                                    lhsT=idsk_sb[:, 0:128],
                                    rhs=p[:, c * 512:(c + 1) * 512],
                                    start=False,
                                    stop=(n == N - 1),
                                )
                        else:
                            # adds on DVE, not GpSimd: concurrent GpSimd
                            # compute halves DVE scan throughput via SBUF
                            # port contention (8.08us vs 4.33us per scan).
                            nc.vector.tensor_tensor(
                                out=acc_sb[:, j * L:(j + 1) * L],
                                in0=acc_sb[:, j * L:(j + 1) * L],
                                in1=p[:, :],
                                op=ALU.add,
                            )

            def phase3(b):
                """silu(z) + gate + out matmul, in 4 time-chunks. Each
                core writes its d_inner-shard partial of [L, DM]; the
                4-way group sum happens on host with the fwd+bwd+bias
                combine (kernel contract allows host gather/unshard).
                yg reuses the xs tile (xs is dead after prep_delta)."""
                s = st[b]
                acc_sb = s["acc"]
                zsil_sb = s["zsil"]
                yg_sb = rpool.tile([128, NB * (L + 3)], DDT, tag="xs")
                s["yg"] = yg_sb
                Q = L // 4          # 512 rows per chunk
                for tc in range(4):
                    c0 = tc * Q
                    for j in range(NB):
                        zs = spool.tile([128, Q], DDT, tag="zs")
                        nc.scalar.activation(
                            out=zs[:, :],
                            in_=zsil_sb[:, j * L + c0:j * L + c0 + Q],
                            func=ACT.Silu,
                        )
                        acc_in = (s["acc0"][tc][:, :] if j == 0 else
                                  acc_sb[:, j * L + c0:j * L + c0 + Q])
                        nc.vector.tensor_tensor(
                            out=yg_sb[:, j * (L + 3) + c0:
                                      j * (L + 3) + c0 + Q],
                            in0=acc_in,
                            in1=zs[:, :],
                            op=ALU.mult,
                        )
                    for tb in range(tc * 4, tc * 4 + 4):
                        for eh in range(2):
                            o_ps = opool.tile([128, 512], F32, tag="omm")
                            for j in range(NB):
                                nc.tensor.matmul(
                                    out=o_ps[:, :],
                                    lhsT=yg_sb[:, j * (L + 3) + tb * 128:
                                               j * (L + 3) + (tb + 1) * 128],
                                    rhs=w_out_sb[:, j * DM + eh * 512:
                                                 j * DM + (eh + 1) * 512],
                                    start=(j == 0),
                                    stop=(j == NB - 1),
                                )
                            o_sb = wpool.tile([128, 512], DDT, tag="osb",
                                              bufs=4)
                            nc.scalar.activation(
                                out=o_sb[:, :], in_=o_ps[:, :], func=ACT.Copy
                            )
                            nc.sync.dma_start(
                                out=out_d.ap()[b * L + tb * 128:
                                               b * L + (tb + 1) * 128,
                                               eh * 512:(eh + 1) * 512],
                                in_=o_sb[:, :],
                            )

            # software pipeline: ALL of b1's prep (conv, x_proj, AllReduce,
            # delta) is emitted inside b0's scan window so the static
            # per-engine streams overlap it with the DVE-bound scan.
            # prep_delta(1) sits after the first 4 scan states so scan(0)'s
            # early dA exps aren't queued behind 60us of ScalarE softplus.
            phase1_mm(0, (0,))
            conv_q(0, 0)
            phase1_mm(0, (1,))
            conv_q(0, 1)
            phase1_rest(0, 0)
            phase1_mm(0, (2,))
            conv_q(0, 2)
            phase1_mm(0, (3,))
            conv_q(0, 3)
            phase1_rest(0, 1)
            prep_delta(0, 0)
            prep_delta(0, 1)
            prep_delta_ln_w(0)
            prep_acc(0)
            phase1_mm(1)
            zrecomp(0)
            scan(0)
            conv_q(1, 0)
            conv_q(1, 1)
            conv_q(1, 2)
            conv_q(1, 3)
            phase1_rest(1, 0)
            phase1_rest(1, 1)
            prep_delta(1, 0)
            prep_delta(1, 1)
            prep_delta_ln_w(1)
            phase3(0)
            prep_acc(1)
            zrecomp(1)
            scan(1)
            phase3(1)

    nc.finalize()
    return nc


def _np_dt(name):
    if name == "bfloat16":
        import ml_dtypes
        return np.dtype(ml_dtypes.bfloat16)
    return np.dtype(np.float32)


def _prep_core_inputs(inputs, core, data_dtype="bfloat16"):
    g = core // 4
    j = core % 4
    rows = slice(j * DS, (j + 1) * DS)
    pref = "fwd_" if g == 0 else "bwd_"
    ddt = _np_dt(data_dtype)

    def P(name):
        return np.asarray(inputs[pref + name], dtype=np.float32)

    x = np.asarray(inputs["x"], dtype=np.float32)
    if g == 1:
        x = x[:, ::-1]
    xT = np.ascontiguousarray(x.transpose(0, 2, 1)).astype(ddt)

    in_proj_w = P("in_proj_w")
    w_xs = np.ascontiguousarray(in_proj_w[rows].T).astype(ddt)
    w_z = np.ascontiguousarray(
        in_proj_w[DI + j * DS:DI + (j + 1) * DS].T
    ).astype(ddt)

    conv_w = P("conv_w")[rows, 0, :]          # [512, 4]
    conv_wd = np.zeros((NB * K_CONV, 128, 128), dtype=np.float32)
    for j in range(NB):
        for k in range(K_CONV):
            np.fill_diagonal(conv_wd[j * K_CONV + k], conv_w[j * 128:(j + 1) * 128, k])
    conv_wd_pack = conv_wd.reshape(NB * K_CONV * 128, 128).astype(ddt)
    conv_b_pack = np.ascontiguousarray(P("conv_b")[rows].reshape(NB, 128).T)
    dtb_pack = np.ascontiguousarray(P("dt_proj_b")[rows].reshape(NB, 128).T)
    dskip_pack = np.ascontiguousarray(P("D")[rows].reshape(NB, 128).T)

    w_xp = np.ascontiguousarray(P("x_proj_w")[:, rows].T).astype(ddt)
    w_dt = np.ascontiguousarray(P("dt_proj_w")[rows].T).astype(ddt)

    A = -np.exp(P("A_log")[rows])             # [512, 16]
    a_pack = np.ascontiguousarray(
        A.reshape(NB, 128, N).transpose(1, 0, 2).reshape(128, NB * N)
    )

    idsk = np.zeros((256, 128), dtype=np.float32)
    np.fill_diagonal(idsk[0:128], 1.0)
    np.fill_diagonal(idsk[128:256], P("D")[rows][0:128])

    fusion_w = np.asarray(inputs["fusion_w"], dtype=np.float32)
    w_out = np.ascontiguousarray(
        P("out_proj_w")[:, rows].T @ fusion_w[:, g * DM:(g + 1) * DM].T
    ).astype(ddt)

    return {
        "xT": xT,
        "w_xs": w_xs,
        "w_z": w_z,
        "w_xp": w_xp,
        "w_dt": w_dt,
        "w_out": w_out,
        "conv_wd": conv_wd_pack,
        "conv_b": conv_b_pack,
        "dtb": dtb_pack,
        "dskip": dskip_pack,
        "a_pack": a_pack,
        "idsk": idsk.astype(ddt),
    }


LAST_EXEC_NS = None


def kernel(**inputs):
    global LAST_EXEC_NS
    from concourse.bass_utils import run_bass_kernel_spmd

    data_dtype = os.environ.get("KERNEL_DATA_DT", "bfloat16")
    scan_dtype = os.environ.get("KERNEL_SCAN_DT", "bfloat16")
    acc_dtype = os.environ.get("KERNEL_ACC_DT", "bfloat16")
    key = (data_dtype, scan_dtype, acc_dtype)
    if key not in _CACHE:
        _CACHE[key] = build_program(data_dtype, scan_dtype, acc_dtype)
    nc = _CACHE[key]

    in_maps = [_prep_core_inputs(inputs, c, data_dtype) for c in range(8)]
    trace = bool(int(os.environ.get("KERNEL_TRACE", "0")))
    res = run_bass_kernel_spmd(nc, in_maps, core_ids=list(range(8)), trace=trace)
    LAST_EXEC_NS = res.exec_time_ns

    shards = [np.asarray(res.results[c]["out"], dtype=np.float32)
              for c in range(8)]
    # each core returns its d_inner-shard partial [B*L, DM]; sum the 4
    # partials per group on host (part of the gather/unshard step).

    def assemble(group):
        tot = shards[group * 4]
        for j in range(1, 4):
            tot = tot + shards[group * 4 + j]
        return tot.reshape(B, L, DM)

    fwd = assemble(0)
    bwd = assemble(1)[:, ::-1]
    fusion_b = np.asarray(inputs["fusion_b"], dtype=np.float32)
    return (fwd + bwd + fusion_b).astype(np.float32)

